# revision 29
# baseline (speedup 1.0000x reference)
"""Size-weighted focal loss on 8 Trainium2 NeuronCores — v4.

Math (per element, x = logit, t in {0,1}):
  w  = x*(1-2t)          so (1-pt) = sigmoid(w)
  N  = softplus(-w)      = ln(1 + e^{-w})
  L  = -log(pt) = softplus(w) = N + w
  s2 = sigmoid(w)^2      = e^{-2N}
  a  = 0.75 - 0.5*t      (alpha_t with ALPHA=0.25)
  elem = a * s2 * L

Device formulation (per core, 4 pairs of samples as [128,4096] tiles,
bf16 intermediates):
  w   = bf16 sign-flip of x in one DVE op: hi16(x) XOR (lo16(t) << 15)
  E   = exp(-w)                                  [ACT]
  N   = ln(E + 1)                                [ACT]
  s2p = exp(-2N + ln 0.5) = 0.5*s2               [ACT]
  F'  = (t - 1.5) * s2p = -(a*s2)                [DVE stt, i32 x bf16]
  PE:  per sample, psum[128,256] += F'^T @ [N | w]  (16 chunks)
  diag extract with mask M[i,i] = M[i,128+i] = -1:
    Scol[:,b] = sum(a*s2*(N+w)) partials per partition-slot
  All activations resolve to natural_log_exp_and_others (see
  _patch_act_tables) -> one ACT_TABLE_LOAD per kernel.

Host: fg_b = count_nonzero(target_b); mean_b( (S_b/HW) * sw(fg_b) ).
"""

import numpy as np
from contextlib import ExitStack

P = 128
B_PER_CORE = 8
GRP = 2                    # samples per tile group
NGRP = B_PER_CORE // GRP
N_CORES = 8
H = 512
W = 512
HW = H * W                 # 262144
FREE = HW // P             # 2048 per sample
GFREE = FREE * GRP         # free dim of a group tile
NCHUNK = FREE // P         # 16 chunks per sample
LN_HALF = -0.6931471805599453

_GLOBAL = {}


def _patch_act_tables():
    """Steer every Exp/Ln activation to the one table set containing both
    (natural_log_exp_and_others), instead of the greedy first-match which
    alternates exp_and_others/natural_log and reloads tables per sample.
    Set order/indices are preserved; only membership is masked."""
    import concourse.bacc as bacc_mod
    import concourse.mybir as mybir
    from concourse.hw_specs import get_activation_tables as _orig

    def _patched(arch):
        A = mybir.ActivationFunctionType
        out = {}
        for name, fns in _orig(arch).items():
            if name != "natural_log_exp_and_others":
                fns = fns - {A.Exp, A.Ln}
            out[name] = fns
        return out

    bacc_mod.get_activation_tables = _patched


def _build():
    import concourse.bacc as bacc
    import concourse.tile as tile
    import concourse.mybir as mybir

    _patch_act_tables()

    f32 = mybir.dt.float32
    i32 = mybir.dt.int32
    bf16 = mybir.dt.bfloat16
    u16 = mybir.dt.uint16
    Alu = mybir.AluOpType
    Act = mybir.ActivationFunctionType

    nc = bacc.Bacc("TRN2", target_bir_lowering=False, debug=False,
                   num_devices=N_CORES)

    pred_in = nc.dram_tensor("pred", (B_PER_CORE, H, W), f32, kind="ExternalInput")
    targ_in = nc.dram_tensor("target", (B_PER_CORE, H, W), i32, kind="ExternalInput")
    mask_in = nc.dram_tensor("mask", (P, 2 * P), f32, kind="ExternalInput")
    out_t = nc.dram_tensor("out", (B_PER_CORE, 1), f32, kind="ExternalOutput")

    # [b, 512, 512] -> [b, 128, 2048]; partition p holds contiguous 2048 elems
    x_v = pred_in.ap().rearrange("b (p q) w -> b p (q w)", p=P)
    t_v = targ_in.ap().rearrange("b (p q) w -> b p (q w)", p=P)

    with ExitStack() as ctx:
        tc = ctx.enter_context(tile.TileContext(nc))
        singles = ctx.enter_context(tc.tile_pool(name="singles", bufs=1))
        io = ctx.enter_context(tc.tile_pool(name="io", bufs=3))
        work = ctx.enter_context(tc.tile_pool(name="work", bufs=2))
        psum = ctx.enter_context(tc.tile_pool(name="psum", bufs=3, space="PSUM"))
        psum_fin = ctx.enter_context(tc.tile_pool(name="psum_fin", bufs=1, space="PSUM"))

        mask_t = singles.tile([P, 2 * P], f32)
        nc.sync.dma_start(out=mask_t[:], in_=mask_in.ap())
        ones_f = singles.tile([P, 1], f32)
        nc.vector.memset(ones_f[:], 1.0)
        shift15_t = singles.tile([P, 1], u16)
        nc.vector.memset(shift15_t[:], 15)
        lnhalf_t = singles.tile([P, 1], f32)
        nc.vector.memset(lnhalf_t[:], LN_HALF)
        Scol = singles.tile([P, B_PER_CORE], f32)   # per-partition loss partials

        # Variable group sizes: singles at the ends for fast pipeline
        # fill/drain, pairs in the middle for ACT pass amortization. Tiles
        # are pair-sized; singles use the left half. Emission is software-
        # pipelined so DVE starts w16(g+1) while ACT runs group g's chain.
        groups = [(0, 1), (1, 2), (3, 2), (5, 2), (7, 1)]
        NG = len(groups)
        st = [dict() for _ in range(NG)]

        def emit_load(g):
            b0, gsz = groups[g]
            xt = io.tile([P, GFREE], f32, tag="xt")
            tt = io.tile([P, GFREE], i32, tag="tt")
            for k in range(gsz):
                sl = slice(k * FREE, (k + 1) * FREE)
                nc.sync.dma_start(out=xt[:, sl], in_=x_v[b0 + k])
                nc.sync.dma_start(out=tt[:, sl], in_=t_v[b0 + k])
            st[g]["xt"], st[g]["tt"] = xt, tt

        def emit_w(g):
            _, gsz = groups[g]
            gf = gsz * FREE
            xt, tt = st[g]["xt"], st[g]["tt"]
            # nw holds both PE rhs blocks: [:,0,:] = N, [:,1,:] = w
            nw = work.tile([P, 2, GFREE], bf16, tag="nw")
            # w = bf16(x), sign flipped where t==1, in one DVE op:
            # (lo16(t) << 15) XOR hi16(x). u16 views keep the ALU integer.
            x_hi = xt[:, :gf].bitcast(u16).rearrange(
                "p (q two) -> p q two", two=2)[:, :, 1]
            t_lo = tt[:, :gf].bitcast(u16).rearrange(
                "p (q two) -> p q two", two=2)[:, :, 0]
            w_u16 = nw[:, 1, :gf].bitcast(u16)
            with tc.high_priority():
                nc.vector.scalar_tensor_tensor(
                    out=w_u16, in0=t_lo, scalar=shift15_t[:], in1=x_hi,
                    op0=Alu.logical_shift_left, op1=Alu.bitwise_xor)
            st[g]["nw"] = nw

        def emit_act(g):
            _, gsz = groups[g]
            gf = gsz * FREE
            nw = st[g]["nw"]
            # eb holds E = exp(-w) first, then is overwritten with
            # s2p = 0.5*s2 (E is dead once N is computed).
            eb = work.tile([P, GFREE], bf16, tag="eb")
            nc.scalar.activation(eb[:, :gf], nw[:, 1, :gf], Act.Exp,
                                 scale=-1.0)
            nc.scalar.activation(nw[:, 0, :gf], eb[:, :gf], Act.Ln, bias=1.0)
            nc.scalar.activation(eb[:, :gf], nw[:, 0, :gf], Act.Exp,
                                 scale=-2.0, bias=lnhalf_t[:])
            st[g]["s2p"] = eb

        def emit_tail(g):
            b0, gsz = groups[g]
            gf = gsz * FREE
            nw, s2p, tt = st[g]["nw"], st[g]["s2p"], st[g]["tt"]
            # F' = (t - 1.5) * 0.5*s2 = -(0.75-0.5t)*s2 = -a*s2
            Ft = work.tile([P, GFREE], bf16, tag="Ft")
            nc.vector.scalar_tensor_tensor(
                out=Ft[:, :gf], in0=tt[:, :gf], scalar=1.5,
                in1=s2p[:, :gf], op0=Alu.subtract, op1=Alu.mult)
            pss = []
            for k in range(gsz):
                ps = psum.tile([P, 2 * P], f32, tag=f"ps{k}")
                for c in range(NCHUNK):
                    sl = slice(k * FREE + c * P, k * FREE + (c + 1) * P)
                    # psum cols 0:128 = F'^T N, cols 128:256 = F'^T w
                    nc.tensor.matmul(ps[:], Ft[:, sl], nw[:, :, sl],
                                     start=(c == 0), stop=(c == NCHUNK - 1))
                pss.append(ps)
            st[g]["pss"] = pss

        def emit_diag(g):
            # Deferred: diag waits on PE; emitting it late keeps the DVE
            # queue from stalling behind it.
            b0, gsz = groups[g]
            for k in range(gsz):
                b = b0 + k
                ps = st[g]["pss"][k]
                scr = work.tile([P, 2 * P], f32, tag=f"scr{k}")
                # Scol[:,b] = sum_j ps[:,j]*mask[:,j] (diag picks -N, -w)
                nc.vector.scalar_tensor_tensor(
                    out=scr[:], in0=ps[:], scalar=0.0, in1=mask_t[:],
                    op0=Alu.add, op1=Alu.mult,
                    accum_out=Scol[:, b:b + 1])

        emit_load(0)
        emit_load(1)
        emit_w(0)
        emit_act(0)
        for g in range(NG):
            if g + 1 < NG:
                emit_w(g + 1)
            emit_tail(g)
            if g >= 1:
                emit_diag(g - 1)
            if g + 2 < NG:
                emit_load(g + 2)
            if g + 1 < NG:
                emit_act(g + 1)
        emit_diag(NG - 1)

        # Final partition reduction via ones-matmul: [128,8]^T @ [128,1] -> [8,1]
        fin = psum_fin.tile([B_PER_CORE, 1], f32)
        nc.tensor.matmul(fin[:, 0:1], Scol[:], ones_f[:], start=True, stop=True)
        out_sb = singles.tile([B_PER_CORE, 1], f32)
        nc.vector.tensor_copy(out_sb[:], fin[:])
        nc.sync.dma_start(out=out_t.ap(), in_=out_sb[:])

    nc.compile()
    return nc


def _get_nc():
    if "nc" not in _GLOBAL:
        _GLOBAL["nc"] = _build()
    return _GLOBAL["nc"]


def _mask_np():
    m = np.zeros((P, 2 * P), dtype=np.float32)
    idx = np.arange(P)
    m[idx, idx] = -1.0          # -(F' * N) = F * N
    m[idx, P + idx] = -1.0      # -(F' * w) = F * w
    return m


GAMMA = 2.0
ALPHA = 0.25
SIZE_POWER = 0.5


def kernel(pred: np.ndarray, target: np.ndarray) -> np.ndarray:
    from concourse import bass_utils

    nc = _get_nc()
    pred = np.ascontiguousarray(np.asarray(pred, dtype=np.float32))
    target = np.ascontiguousarray(np.asarray(target, dtype=np.int32))
    mask = _mask_np()

    in_maps = []
    for i in range(N_CORES):
        sl = slice(i * B_PER_CORE, (i + 1) * B_PER_CORE)
        in_maps.append({
            "pred": np.ascontiguousarray(pred[sl, 0]),
            "target": np.ascontiguousarray(target[sl]),
            "mask": mask,
        })

    res = bass_utils.run_bass_kernel_spmd(
        nc, in_maps, core_ids=list(range(N_CORES)),
        trace=bool(_GLOBAL.get("trace", False)),
        **_GLOBAL.get("run_kwargs", {}),
    )
    _GLOBAL["last_results"] = res

    outs = np.concatenate([r["out"] for r in res.results], axis=0)  # [64, 1]
    S = outs[:, 0].astype(np.float64)          # per-sample sum(a*s2*L)
    fg = np.count_nonzero(target.reshape(target.shape[0], -1), axis=1)
    fg = fg.astype(np.float64)
    sw = np.where(fg > 0,
                  np.minimum(100.0 / np.power(np.maximum(fg, 1.0), SIZE_POWER), 10.0),
                  1.0)
    per_sample = (S / HW) * sw
    return np.float32(per_sample.mean())


# revision 33
# speedup vs baseline: 1.0553x; 1.0553x over previous
"""Size-weighted focal loss on 8 Trainium2 NeuronCores — v4.

Math (per element, x = logit, t in {0,1}):
  w  = x*(1-2t)          so (1-pt) = sigmoid(w)
  N  = softplus(-w)      = ln(1 + e^{-w})
  L  = -log(pt) = softplus(w) = N + w
  s2 = sigmoid(w)^2      = e^{-2N}
  a  = 0.75 - 0.5*t      (alpha_t with ALPHA=0.25)
  elem = a * s2 * L

Device formulation (per core, 4 pairs of samples as [128,4096] tiles,
bf16 intermediates):
  w   = bf16 sign-flip of x in one DVE op: hi16(x) XOR (lo16(t) << 15)
  E   = exp(-w)                                  [ACT]
  N   = ln(E + 1)                                [ACT]
  s2p = exp(-2N + ln 0.5) = 0.5*s2               [ACT]
  F'  = (t - 1.5) * s2p = -(a*s2)                [DVE stt, i32 x bf16]
  PE:  per sample, psum[128,256] += F'^T @ [N | w]  (16 chunks)
  diag extract with mask M[i,i] = M[i,128+i] = -1:
    Scol[:,b] = sum(a*s2*(N+w)) partials per partition-slot
  All activations resolve to natural_log_exp_and_others (see
  _patch_act_tables) -> one ACT_TABLE_LOAD per kernel.

Host: fg_b = count_nonzero(target_b); mean_b( (S_b/HW) * sw(fg_b) ).
"""

import numpy as np
from contextlib import ExitStack

P = 128
B_PER_CORE = 8
GRP = 2                    # samples per tile group
NGRP = B_PER_CORE // GRP
N_CORES = 8
H = 512
W = 512
HW = H * W                 # 262144
FREE = HW // P             # 2048 per sample
GFREE = FREE * GRP         # free dim of a group tile
NCHUNK = FREE // P         # 16 chunks per sample
LN_HALF = -0.6931471805599453

_GLOBAL = {}


def _patch_act_tables():
    """Steer every Exp/Ln activation to the one table set containing both
    (natural_log_exp_and_others), instead of the greedy first-match which
    alternates exp_and_others/natural_log and reloads tables per sample.
    Set order/indices are preserved; only membership is masked."""
    import concourse.bacc as bacc_mod
    import concourse.mybir as mybir
    from concourse.hw_specs import get_activation_tables as _orig

    def _patched(arch):
        A = mybir.ActivationFunctionType
        out = {}
        for name, fns in _orig(arch).items():
            if name != "natural_log_exp_and_others":
                fns = fns - {A.Exp, A.Ln}
            out[name] = fns
        return out

    bacc_mod.get_activation_tables = _patched


def _build():
    import concourse.bacc as bacc
    import concourse.tile as tile
    import concourse.mybir as mybir

    _patch_act_tables()

    f32 = mybir.dt.float32
    i32 = mybir.dt.int32
    bf16 = mybir.dt.bfloat16
    u16 = mybir.dt.uint16
    Alu = mybir.AluOpType
    Act = mybir.ActivationFunctionType

    nc = bacc.Bacc("TRN2", target_bir_lowering=False, debug=False,
                   num_devices=N_CORES)

    pred_in = nc.dram_tensor("pred", (B_PER_CORE, H, W), f32, kind="ExternalInput")
    targ_in = nc.dram_tensor("target", (B_PER_CORE, H, W), i32, kind="ExternalInput")
    mask_in = nc.dram_tensor("mask", (P, 2 * P), f32, kind="ExternalInput")
    out_t = nc.dram_tensor("out", (B_PER_CORE, 1), f32, kind="ExternalOutput")

    # [b, 512, 512] -> [b, 128, 2048]; partition p holds contiguous 2048 elems
    x_v = pred_in.ap().rearrange("b (p q) w -> b p (q w)", p=P)
    t_v = targ_in.ap().rearrange("b (p q) w -> b p (q w)", p=P)

    with ExitStack() as ctx:
        tc = ctx.enter_context(tile.TileContext(nc))
        singles = ctx.enter_context(tc.tile_pool(name="singles", bufs=1))
        io = ctx.enter_context(tc.tile_pool(name="io", bufs=3))
        work = ctx.enter_context(tc.tile_pool(name="work", bufs=2))
        psum = ctx.enter_context(tc.tile_pool(name="psum", bufs=3, space="PSUM"))
        psum_fin = ctx.enter_context(tc.tile_pool(name="psum_fin", bufs=1, space="PSUM"))

        mask_t = singles.tile([P, 2 * P], f32)
        nc.sync.dma_start(out=mask_t[:], in_=mask_in.ap())
        ones_f = singles.tile([P, 1], f32)
        nc.vector.memset(ones_f[:], 1.0)
        shift15_t = singles.tile([P, 1], u16)
        nc.vector.memset(shift15_t[:], 15)
        lnhalf_t = singles.tile([P, 1], f32)
        nc.vector.memset(lnhalf_t[:], LN_HALF)
        Scol = singles.tile([P, B_PER_CORE], f32)   # per-partition loss partials

        # Variable group sizes: singles at the ends for fast pipeline
        # fill/drain, pairs in the middle for ACT pass amortization. Tiles
        # are pair-sized; singles use the left half. Emission is software-
        # pipelined so DVE starts w16(g+1) while ACT runs group g's chain.
        groups = [(0, 1), (1, 2), (3, 2), (5, 2), (7, 1)]
        NG = len(groups)
        st = [dict() for _ in range(NG)]

        def emit_load(g):
            b0, gsz = groups[g]
            xt = io.tile([P, GFREE], f32, tag="xt")
            tt = io.tile([P, GFREE], i32, tag="tt")
            for k in range(gsz):
                sl = slice(k * FREE, (k + 1) * FREE)
                nc.sync.dma_start(out=xt[:, sl], in_=x_v[b0 + k])
                nc.sync.dma_start(out=tt[:, sl], in_=t_v[b0 + k])
            st[g]["xt"], st[g]["tt"] = xt, tt

        def emit_w(g):
            _, gsz = groups[g]
            gf = gsz * FREE
            xt, tt = st[g]["xt"], st[g]["tt"]
            # nw holds both PE rhs blocks: [:,0,:] = N, [:,1,:] = w
            nw = work.tile([P, 2, GFREE], bf16, tag="nw")
            # w = bf16(x), sign flipped where t==1, in one DVE op:
            # (lo16(t) << 15) XOR hi16(x). u16 views keep the ALU integer.
            x_hi = xt[:, :gf].bitcast(u16).rearrange(
                "p (q two) -> p q two", two=2)[:, :, 1]
            t_lo = tt[:, :gf].bitcast(u16).rearrange(
                "p (q two) -> p q two", two=2)[:, :, 0]
            w_u16 = nw[:, 1, :gf].bitcast(u16)
            winst = nc.vector.scalar_tensor_tensor(
                out=w_u16, in0=t_lo, scalar=shift15_t[:], in1=x_hi,
                op0=Alu.logical_shift_left, op1=Alu.bitwise_xor)
            st[g]["nw"] = nw
            st[g]["winst"] = winst

        def emit_act(g):
            _, gsz = groups[g]
            gf = gsz * FREE
            nw = st[g]["nw"]
            # eb holds E = exp(-w) first, then is overwritten with
            # s2p = 0.5*s2 (E is dead once N is computed).
            eb = work.tile([P, GFREE], bf16, tag="eb")
            nc.scalar.activation(eb[:, :gf], nw[:, 1, :gf], Act.Exp,
                                 scale=-1.0)
            nc.scalar.activation(nw[:, 0, :gf], eb[:, :gf], Act.Ln, bias=1.0)
            nc.scalar.activation(eb[:, :gf], nw[:, 0, :gf], Act.Exp,
                                 scale=-2.0, bias=lnhalf_t[:])
            st[g]["s2p"] = eb

        def emit_tail(g):
            b0, gsz = groups[g]
            gf = gsz * FREE
            nw, s2p, tt = st[g]["nw"], st[g]["s2p"], st[g]["tt"]
            # F' = (t - 1.5) * 0.5*s2 = -(0.75-0.5t)*s2 = -a*s2
            Ft = work.tile([P, GFREE], bf16, tag="Ft")
            finst = nc.vector.scalar_tensor_tensor(
                out=Ft[:, :gf], in0=tt[:, :gf], scalar=1.5,
                in1=s2p[:, :gf], op0=Alu.subtract, op1=Alu.mult)
            # Order DVE so the next group's w16 (data-ready early) runs
            # before this F' (which waits on the ACT chain).
            if g + 1 < NG and "winst" in st[g + 1]:
                import bass_rust as _br
                finst.ins.add_nosync_dependencies_from(
                    _br.InstructionNameOrderedSet([st[g + 1]["winst"].ins.name]))
            pss = []
            for k in range(gsz):
                ps = psum.tile([P, 2 * P], f32, tag=f"ps{k}")
                for c in range(NCHUNK):
                    sl = slice(k * FREE + c * P, k * FREE + (c + 1) * P)
                    # psum cols 0:128 = F'^T N, cols 128:256 = F'^T w
                    nc.tensor.matmul(ps[:], Ft[:, sl], nw[:, :, sl],
                                     start=(c == 0), stop=(c == NCHUNK - 1))
                pss.append(ps)
            st[g]["pss"] = pss

        def emit_diag(g):
            # Deferred: diag waits on PE; emitting it late keeps the DVE
            # queue from stalling behind it.
            b0, gsz = groups[g]
            for k in range(gsz):
                b = b0 + k
                ps = st[g]["pss"][k]
                scr = work.tile([P, 2 * P], f32, tag=f"scr{k}")
                # Scol[:,b] = sum_j ps[:,j]*mask[:,j] (diag picks -N, -w)
                nc.vector.scalar_tensor_tensor(
                    out=scr[:], in0=ps[:], scalar=0.0, in1=mask_t[:],
                    op0=Alu.add, op1=Alu.mult,
                    accum_out=Scol[:, b:b + 1])

        emit_load(0)
        emit_load(1)
        emit_w(0)
        emit_act(0)
        for g in range(NG):
            if g + 1 < NG:
                emit_w(g + 1)
            emit_tail(g)
            if g >= 1:
                emit_diag(g - 1)
            if g + 2 < NG:
                emit_load(g + 2)
            if g + 1 < NG:
                emit_act(g + 1)
        emit_diag(NG - 1)

        # Final partition reduction via ones-matmul: [128,8]^T @ [128,1] -> [8,1]
        fin = psum_fin.tile([B_PER_CORE, 1], f32)
        nc.tensor.matmul(fin[:, 0:1], Scol[:], ones_f[:], start=True, stop=True)
        out_sb = singles.tile([B_PER_CORE, 1], f32)
        nc.vector.tensor_copy(out_sb[:], fin[:])
        nc.sync.dma_start(out=out_t.ap(), in_=out_sb[:])

    nc.compile()
    return nc


def _get_nc():
    if "nc" not in _GLOBAL:
        _GLOBAL["nc"] = _build()
    return _GLOBAL["nc"]


def _mask_np():
    m = np.zeros((P, 2 * P), dtype=np.float32)
    idx = np.arange(P)
    m[idx, idx] = -1.0          # -(F' * N) = F * N
    m[idx, P + idx] = -1.0      # -(F' * w) = F * w
    return m


GAMMA = 2.0
ALPHA = 0.25
SIZE_POWER = 0.5


def kernel(pred: np.ndarray, target: np.ndarray) -> np.ndarray:
    from concourse import bass_utils

    nc = _get_nc()
    pred = np.ascontiguousarray(np.asarray(pred, dtype=np.float32))
    target = np.ascontiguousarray(np.asarray(target, dtype=np.int32))
    mask = _mask_np()

    in_maps = []
    for i in range(N_CORES):
        sl = slice(i * B_PER_CORE, (i + 1) * B_PER_CORE)
        in_maps.append({
            "pred": np.ascontiguousarray(pred[sl, 0]),
            "target": np.ascontiguousarray(target[sl]),
            "mask": mask,
        })

    res = bass_utils.run_bass_kernel_spmd(
        nc, in_maps, core_ids=list(range(N_CORES)),
        trace=bool(_GLOBAL.get("trace", False)),
        **_GLOBAL.get("run_kwargs", {}),
    )
    _GLOBAL["last_results"] = res

    outs = np.concatenate([r["out"] for r in res.results], axis=0)  # [64, 1]
    S = outs[:, 0].astype(np.float64)          # per-sample sum(a*s2*L)
    fg = np.count_nonzero(target.reshape(target.shape[0], -1), axis=1)
    fg = fg.astype(np.float64)
    sw = np.where(fg > 0,
                  np.minimum(100.0 / np.power(np.maximum(fg, 1.0), SIZE_POWER), 10.0),
                  1.0)
    per_sample = (S / HW) * sw
    return np.float32(per_sample.mean())


# revision 34
# speedup vs baseline: 1.0775x; 1.0211x over previous
"""Size-weighted focal loss on 8 Trainium2 NeuronCores — v4.

Math (per element, x = logit, t in {0,1}):
  w  = x*(1-2t)          so (1-pt) = sigmoid(w)
  N  = softplus(-w)      = ln(1 + e^{-w})
  L  = -log(pt) = softplus(w) = N + w
  s2 = sigmoid(w)^2      = e^{-2N}
  a  = 0.75 - 0.5*t      (alpha_t with ALPHA=0.25)
  elem = a * s2 * L

Device formulation (per core, 4 pairs of samples as [128,4096] tiles,
bf16 intermediates):
  w   = bf16 sign-flip of x in one DVE op: hi16(x) XOR (lo16(t) << 15)
  E   = exp(-w)                                  [ACT]
  N   = ln(E + 1)                                [ACT]
  s2p = exp(-2N + ln 0.5) = 0.5*s2               [ACT]
  F'  = (t - 1.5) * s2p = -(a*s2)                [DVE stt, i32 x bf16]
  PE:  per sample, psum[128,256] += F'^T @ [N | w]  (16 chunks)
  diag extract with mask M[i,i] = M[i,128+i] = -1:
    Scol[:,b] = sum(a*s2*(N+w)) partials per partition-slot
  All activations resolve to natural_log_exp_and_others (see
  _patch_act_tables) -> one ACT_TABLE_LOAD per kernel.

Host: fg_b = count_nonzero(target_b); mean_b( (S_b/HW) * sw(fg_b) ).
"""

import numpy as np
from contextlib import ExitStack

P = 128
B_PER_CORE = 8
GRP = 2                    # samples per tile group
NGRP = B_PER_CORE // GRP
N_CORES = 8
H = 512
W = 512
HW = H * W                 # 262144
FREE = HW // P             # 2048 per sample
GFREE = FREE * GRP         # free dim of a group tile
NCHUNK = FREE // P         # 16 chunks per sample
LN_HALF = -0.6931471805599453

_GLOBAL = {}


def _patch_act_tables():
    """Steer every Exp/Ln activation to the one table set containing both
    (natural_log_exp_and_others), instead of the greedy first-match which
    alternates exp_and_others/natural_log and reloads tables per sample.
    Set order/indices are preserved; only membership is masked."""
    import concourse.bacc as bacc_mod
    import concourse.mybir as mybir
    from concourse.hw_specs import get_activation_tables as _orig

    def _patched(arch):
        A = mybir.ActivationFunctionType
        out = {}
        for name, fns in _orig(arch).items():
            if name != "natural_log_exp_and_others":
                fns = fns - {A.Exp, A.Ln}
            out[name] = fns
        return out

    bacc_mod.get_activation_tables = _patched


def _build():
    import concourse.bacc as bacc
    import concourse.tile as tile
    import concourse.mybir as mybir

    _patch_act_tables()

    f32 = mybir.dt.float32
    i32 = mybir.dt.int32
    bf16 = mybir.dt.bfloat16
    u16 = mybir.dt.uint16
    Alu = mybir.AluOpType
    Act = mybir.ActivationFunctionType

    nc = bacc.Bacc("TRN2", target_bir_lowering=False, debug=False,
                   num_devices=N_CORES)

    pred_in = nc.dram_tensor("pred", (B_PER_CORE, H, W), f32, kind="ExternalInput")
    targ_in = nc.dram_tensor("target", (B_PER_CORE, H, W), i32, kind="ExternalInput")
    mask_in = nc.dram_tensor("mask", (P, 2 * P), f32, kind="ExternalInput")
    out_t = nc.dram_tensor("out", (B_PER_CORE, 1), f32, kind="ExternalOutput")

    # [b, 512, 512] -> [b, 128, 2048]; partition p holds contiguous 2048 elems
    x_v = pred_in.ap().rearrange("b (p q) w -> b p (q w)", p=P)
    t_v = targ_in.ap().rearrange("b (p q) w -> b p (q w)", p=P)

    with ExitStack() as ctx:
        tc = ctx.enter_context(tile.TileContext(nc))
        singles = ctx.enter_context(tc.tile_pool(name="singles", bufs=1))
        io = ctx.enter_context(tc.tile_pool(name="io", bufs=3))
        work = ctx.enter_context(tc.tile_pool(name="work", bufs=2))
        nwpool = ctx.enter_context(tc.tile_pool(name="nwpool", bufs=3))
        psum = ctx.enter_context(tc.tile_pool(name="psum", bufs=3, space="PSUM"))
        psum_fin = ctx.enter_context(tc.tile_pool(name="psum_fin", bufs=1, space="PSUM"))

        mask_t = singles.tile([P, 2 * P], f32)
        nc.sync.dma_start(out=mask_t[:], in_=mask_in.ap())
        ones_f = singles.tile([P, 1], f32)
        nc.vector.memset(ones_f[:], 1.0)
        shift15_t = singles.tile([P, 1], u16)
        nc.vector.memset(shift15_t[:], 15)
        lnhalf_t = singles.tile([P, 1], f32)
        nc.vector.memset(lnhalf_t[:], LN_HALF)
        Scol = singles.tile([P, B_PER_CORE], f32)   # per-partition loss partials

        # Variable group sizes: singles at the ends for fast pipeline
        # fill/drain, pairs in the middle for ACT pass amortization. Tiles
        # are pair-sized; singles use the left half. Emission is software-
        # pipelined so DVE starts w16(g+1) while ACT runs group g's chain.
        groups = [(0, 1), (1, 2), (3, 2), (5, 2), (7, 1)]
        NG = len(groups)
        st = [dict() for _ in range(NG)]

        def emit_load(g):
            b0, gsz = groups[g]
            xt = io.tile([P, GFREE], f32, tag="xt")
            tt = io.tile([P, GFREE], i32, tag="tt")
            for k in range(gsz):
                sl = slice(k * FREE, (k + 1) * FREE)
                nc.sync.dma_start(out=xt[:, sl], in_=x_v[b0 + k])
                nc.sync.dma_start(out=tt[:, sl], in_=t_v[b0 + k])
            st[g]["xt"], st[g]["tt"] = xt, tt

        def emit_w(g):
            _, gsz = groups[g]
            gf = gsz * FREE
            xt, tt = st[g]["xt"], st[g]["tt"]
            # nw holds both PE rhs blocks: [:,0,:] = N, [:,1,:] = w
            nw = nwpool.tile([P, 2, GFREE], bf16, tag="nw")
            # w = bf16(x), sign flipped where t==1, in one DVE op:
            # (lo16(t) << 15) XOR hi16(x). u16 views keep the ALU integer.
            x_hi = xt[:, :gf].bitcast(u16).rearrange(
                "p (q two) -> p q two", two=2)[:, :, 1]
            t_lo = tt[:, :gf].bitcast(u16).rearrange(
                "p (q two) -> p q two", two=2)[:, :, 0]
            w_u16 = nw[:, 1, :gf].bitcast(u16)
            winst = nc.vector.scalar_tensor_tensor(
                out=w_u16, in0=t_lo, scalar=shift15_t[:], in1=x_hi,
                op0=Alu.logical_shift_left, op1=Alu.bitwise_xor)
            st[g]["nw"] = nw
            st[g]["winst"] = winst

        def emit_act(g):
            _, gsz = groups[g]
            gf = gsz * FREE
            nw = st[g]["nw"]
            # eb holds E = exp(-w) first, then is overwritten with
            # s2p = 0.5*s2 (E is dead once N is computed).
            eb = work.tile([P, GFREE], bf16, tag="eb")
            nc.scalar.activation(eb[:, :gf], nw[:, 1, :gf], Act.Exp,
                                 scale=-1.0)
            nc.scalar.activation(nw[:, 0, :gf], eb[:, :gf], Act.Ln, bias=1.0)
            nc.scalar.activation(eb[:, :gf], nw[:, 0, :gf], Act.Exp,
                                 scale=-2.0, bias=lnhalf_t[:])
            st[g]["s2p"] = eb

        def emit_tail(g):
            b0, gsz = groups[g]
            gf = gsz * FREE
            nw, s2p, tt = st[g]["nw"], st[g]["s2p"], st[g]["tt"]
            # F' = (t - 1.5) * 0.5*s2 = -(0.75-0.5t)*s2 = -a*s2
            Ft = work.tile([P, GFREE], bf16, tag="Ft")
            finst = nc.vector.scalar_tensor_tensor(
                out=Ft[:, :gf], in0=tt[:, :gf], scalar=1.5,
                in1=s2p[:, :gf], op0=Alu.subtract, op1=Alu.mult)
            # Order DVE so the next group's w16 (data-ready early) runs
            # before this F' (which waits on the ACT chain).
            if g + 1 < NG and "winst" in st[g + 1]:
                import bass_rust as _br
                finst.ins.add_nosync_dependencies_from(
                    _br.InstructionNameOrderedSet([st[g + 1]["winst"].ins.name]))
            pss = []
            for k in range(gsz):
                ps = psum.tile([P, 2 * P], f32, tag=f"ps{k}")
                for c in range(NCHUNK):
                    sl = slice(k * FREE + c * P, k * FREE + (c + 1) * P)
                    # psum cols 0:128 = F'^T N, cols 128:256 = F'^T w
                    nc.tensor.matmul(ps[:], Ft[:, sl], nw[:, :, sl],
                                     start=(c == 0), stop=(c == NCHUNK - 1))
                pss.append(ps)
            st[g]["pss"] = pss

        def emit_diag(g):
            # Deferred: diag waits on PE; emitting it late keeps the DVE
            # queue from stalling behind it.
            b0, gsz = groups[g]
            for k in range(gsz):
                b = b0 + k
                ps = st[g]["pss"][k]
                scr = work.tile([P, 2 * P], f32, tag=f"scr{k}")
                # Scol[:,b] = sum_j ps[:,j]*mask[:,j] (diag picks -N, -w)
                nc.vector.scalar_tensor_tensor(
                    out=scr[:], in0=ps[:], scalar=0.0, in1=mask_t[:],
                    op0=Alu.add, op1=Alu.mult,
                    accum_out=Scol[:, b:b + 1])

        emit_load(0)
        emit_load(1)
        emit_w(0)
        emit_act(0)
        for g in range(NG):
            if g + 1 < NG:
                emit_w(g + 1)
            emit_tail(g)
            if g >= 1:
                emit_diag(g - 1)
            if g + 2 < NG:
                emit_load(g + 2)
            if g + 1 < NG:
                emit_act(g + 1)
        emit_diag(NG - 1)

        # Final partition reduction via ones-matmul: [128,8]^T @ [128,1] -> [8,1]
        fin = psum_fin.tile([B_PER_CORE, 1], f32)
        nc.tensor.matmul(fin[:, 0:1], Scol[:], ones_f[:], start=True, stop=True)
        out_sb = singles.tile([B_PER_CORE, 1], f32)
        nc.vector.tensor_copy(out_sb[:], fin[:])
        nc.sync.dma_start(out=out_t.ap(), in_=out_sb[:])

    nc.compile()
    return nc


def _get_nc():
    if "nc" not in _GLOBAL:
        _GLOBAL["nc"] = _build()
    return _GLOBAL["nc"]


def _mask_np():
    m = np.zeros((P, 2 * P), dtype=np.float32)
    idx = np.arange(P)
    m[idx, idx] = -1.0          # -(F' * N) = F * N
    m[idx, P + idx] = -1.0      # -(F' * w) = F * w
    return m


GAMMA = 2.0
ALPHA = 0.25
SIZE_POWER = 0.5


def kernel(pred: np.ndarray, target: np.ndarray) -> np.ndarray:
    from concourse import bass_utils

    nc = _get_nc()
    pred = np.ascontiguousarray(np.asarray(pred, dtype=np.float32))
    target = np.ascontiguousarray(np.asarray(target, dtype=np.int32))
    mask = _mask_np()

    in_maps = []
    for i in range(N_CORES):
        sl = slice(i * B_PER_CORE, (i + 1) * B_PER_CORE)
        in_maps.append({
            "pred": np.ascontiguousarray(pred[sl, 0]),
            "target": np.ascontiguousarray(target[sl]),
            "mask": mask,
        })

    res = bass_utils.run_bass_kernel_spmd(
        nc, in_maps, core_ids=list(range(N_CORES)),
        trace=bool(_GLOBAL.get("trace", False)),
        **_GLOBAL.get("run_kwargs", {}),
    )
    _GLOBAL["last_results"] = res

    outs = np.concatenate([r["out"] for r in res.results], axis=0)  # [64, 1]
    S = outs[:, 0].astype(np.float64)          # per-sample sum(a*s2*L)
    fg = np.count_nonzero(target.reshape(target.shape[0], -1), axis=1)
    fg = fg.astype(np.float64)
    sw = np.where(fg > 0,
                  np.minimum(100.0 / np.power(np.maximum(fg, 1.0), SIZE_POWER), 10.0),
                  1.0)
    per_sample = (S / HW) * sw
    return np.float32(per_sample.mean())


# revision 35
# speedup vs baseline: 1.0781x; 1.0005x over previous
"""Size-weighted focal loss on 8 Trainium2 NeuronCores — v4.

Math (per element, x = logit, t in {0,1}):
  w  = x*(1-2t)          so (1-pt) = sigmoid(w)
  N  = softplus(-w)      = ln(1 + e^{-w})
  L  = -log(pt) = softplus(w) = N + w
  s2 = sigmoid(w)^2      = e^{-2N}
  a  = 0.75 - 0.5*t      (alpha_t with ALPHA=0.25)
  elem = a * s2 * L

Device formulation (per core, 4 pairs of samples as [128,4096] tiles,
bf16 intermediates):
  w   = bf16 sign-flip of x in one DVE op: hi16(x) XOR (lo16(t) << 15)
  E   = exp(-w)                                  [ACT]
  N   = ln(E + 1)                                [ACT]
  s2p = exp(-2N + ln 0.5) = 0.5*s2               [ACT]
  F'  = (t - 1.5) * s2p = -(a*s2)                [DVE stt, i32 x bf16]
  PE:  per sample, psum[128,256] += F'^T @ [N | w]  (16 chunks)
  diag extract with mask M[i,i] = M[i,128+i] = -1:
    Scol[:,b] = sum(a*s2*(N+w)) partials per partition-slot
  All activations resolve to natural_log_exp_and_others (see
  _patch_act_tables) -> one ACT_TABLE_LOAD per kernel.

Host: fg_b = count_nonzero(target_b); mean_b( (S_b/HW) * sw(fg_b) ).
"""

import numpy as np
from contextlib import ExitStack

P = 128
B_PER_CORE = 8
GRP = 2                    # samples per tile group
NGRP = B_PER_CORE // GRP
N_CORES = 8
H = 512
W = 512
HW = H * W                 # 262144
FREE = HW // P             # 2048 per sample
GFREE = FREE * GRP         # free dim of a group tile
NCHUNK = FREE // P         # 16 chunks per sample
LN_HALF = -0.6931471805599453

_GLOBAL = {}


def _patch_act_tables():
    """Steer every Exp/Ln activation to the one table set containing both
    (natural_log_exp_and_others), instead of the greedy first-match which
    alternates exp_and_others/natural_log and reloads tables per sample.
    Set order/indices are preserved; only membership is masked."""
    import concourse.bacc as bacc_mod
    import concourse.mybir as mybir
    from concourse.hw_specs import get_activation_tables as _orig

    def _patched(arch):
        A = mybir.ActivationFunctionType
        out = {}
        for name, fns in _orig(arch).items():
            if name != "natural_log_exp_and_others":
                fns = fns - {A.Exp, A.Ln}
            out[name] = fns
        return out

    bacc_mod.get_activation_tables = _patched


def _build():
    import concourse.bacc as bacc
    import concourse.tile as tile
    import concourse.mybir as mybir

    _patch_act_tables()

    f32 = mybir.dt.float32
    i32 = mybir.dt.int32
    bf16 = mybir.dt.bfloat16
    u16 = mybir.dt.uint16
    Alu = mybir.AluOpType
    Act = mybir.ActivationFunctionType

    nc = bacc.Bacc("TRN2", target_bir_lowering=False, debug=False,
                   num_devices=N_CORES)

    pred_in = nc.dram_tensor("pred", (B_PER_CORE, H, W), f32, kind="ExternalInput")
    targ_in = nc.dram_tensor("target", (B_PER_CORE, H, W), i32, kind="ExternalInput")
    mask_in = nc.dram_tensor("mask", (P, 2 * P), f32, kind="ExternalInput")
    out_t = nc.dram_tensor("out", (P, B_PER_CORE), f32, kind="ExternalOutput")

    # [b, 512, 512] -> [b, 128, 2048]; partition p holds contiguous 2048 elems
    x_v = pred_in.ap().rearrange("b (p q) w -> b p (q w)", p=P)
    t_v = targ_in.ap().rearrange("b (p q) w -> b p (q w)", p=P)

    with ExitStack() as ctx:
        tc = ctx.enter_context(tile.TileContext(nc))
        singles = ctx.enter_context(tc.tile_pool(name="singles", bufs=1))
        io = ctx.enter_context(tc.tile_pool(name="io", bufs=3))
        work = ctx.enter_context(tc.tile_pool(name="work", bufs=2))
        nwpool = ctx.enter_context(tc.tile_pool(name="nwpool", bufs=3))
        psum = ctx.enter_context(tc.tile_pool(name="psum", bufs=3, space="PSUM"))

        mask_t = singles.tile([P, 2 * P], f32)
        nc.sync.dma_start(out=mask_t[:], in_=mask_in.ap())
        shift15_t = singles.tile([P, 1], u16)
        nc.vector.memset(shift15_t[:], 15)
        lnhalf_t = singles.tile([P, 1], f32)
        nc.vector.memset(lnhalf_t[:], LN_HALF)
        Scol = singles.tile([P, B_PER_CORE], f32)   # per-partition loss partials

        # Variable group sizes: singles at the ends for fast pipeline
        # fill/drain, pairs in the middle for ACT pass amortization. Tiles
        # are pair-sized; singles use the left half. Emission is software-
        # pipelined so DVE starts w16(g+1) while ACT runs group g's chain.
        groups = [(0, 1), (1, 2), (3, 2), (5, 2), (7, 1)]
        NG = len(groups)
        st = [dict() for _ in range(NG)]

        def emit_load(g):
            b0, gsz = groups[g]
            xt = io.tile([P, GFREE], f32, tag="xt")
            tt = io.tile([P, GFREE], i32, tag="tt")
            for k in range(gsz):
                sl = slice(k * FREE, (k + 1) * FREE)
                nc.sync.dma_start(out=xt[:, sl], in_=x_v[b0 + k])
                nc.sync.dma_start(out=tt[:, sl], in_=t_v[b0 + k])
            st[g]["xt"], st[g]["tt"] = xt, tt

        def emit_w(g):
            _, gsz = groups[g]
            gf = gsz * FREE
            xt, tt = st[g]["xt"], st[g]["tt"]
            # nw holds both PE rhs blocks: [:,0,:] = N, [:,1,:] = w
            nw = nwpool.tile([P, 2, GFREE], bf16, tag="nw")
            # w = bf16(x), sign flipped where t==1, in one DVE op:
            # (lo16(t) << 15) XOR hi16(x). u16 views keep the ALU integer.
            x_hi = xt[:, :gf].bitcast(u16).rearrange(
                "p (q two) -> p q two", two=2)[:, :, 1]
            t_lo = tt[:, :gf].bitcast(u16).rearrange(
                "p (q two) -> p q two", two=2)[:, :, 0]
            w_u16 = nw[:, 1, :gf].bitcast(u16)
            winst = nc.vector.scalar_tensor_tensor(
                out=w_u16, in0=t_lo, scalar=shift15_t[:], in1=x_hi,
                op0=Alu.logical_shift_left, op1=Alu.bitwise_xor)
            st[g]["nw"] = nw
            st[g]["winst"] = winst

        def emit_act(g):
            _, gsz = groups[g]
            gf = gsz * FREE
            nw = st[g]["nw"]
            # eb holds E = exp(-w) first, then is overwritten with
            # s2p = 0.5*s2 (E is dead once N is computed).
            eb = work.tile([P, GFREE], bf16, tag="eb")
            nc.scalar.activation(eb[:, :gf], nw[:, 1, :gf], Act.Exp,
                                 scale=-1.0)
            nc.scalar.activation(nw[:, 0, :gf], eb[:, :gf], Act.Ln, bias=1.0)
            nc.scalar.activation(eb[:, :gf], nw[:, 0, :gf], Act.Exp,
                                 scale=-2.0, bias=lnhalf_t[:])
            st[g]["s2p"] = eb

        def emit_tail(g):
            b0, gsz = groups[g]
            gf = gsz * FREE
            nw, s2p, tt = st[g]["nw"], st[g]["s2p"], st[g]["tt"]
            # F' = (t - 1.5) * 0.5*s2 = -(0.75-0.5t)*s2 = -a*s2
            Ft = work.tile([P, GFREE], bf16, tag="Ft")
            finst = nc.vector.scalar_tensor_tensor(
                out=Ft[:, :gf], in0=tt[:, :gf], scalar=1.5,
                in1=s2p[:, :gf], op0=Alu.subtract, op1=Alu.mult)
            # Order DVE so the next group's w16 (data-ready early) runs
            # before this F' (which waits on the ACT chain).
            if g + 1 < NG and "winst" in st[g + 1]:
                import bass_rust as _br
                finst.ins.add_nosync_dependencies_from(
                    _br.InstructionNameOrderedSet([st[g + 1]["winst"].ins.name]))
            pss = []
            for k in range(gsz):
                ps = psum.tile([P, 2 * P], f32, tag=f"ps{k}")
                for c in range(NCHUNK):
                    sl = slice(k * FREE + c * P, k * FREE + (c + 1) * P)
                    # psum cols 0:128 = F'^T N, cols 128:256 = F'^T w
                    nc.tensor.matmul(ps[:], Ft[:, sl], nw[:, :, sl],
                                     start=(c == 0), stop=(c == NCHUNK - 1))
                pss.append(ps)
            st[g]["pss"] = pss

        def emit_diag(g):
            # Deferred: diag waits on PE; emitting it late keeps the DVE
            # queue from stalling behind it.
            b0, gsz = groups[g]
            for k in range(gsz):
                b = b0 + k
                ps = st[g]["pss"][k]
                scr = work.tile([P, 2 * P], f32, tag=f"scr{k}")
                # Scol[:,b] = sum_j ps[:,j]*mask[:,j] (diag picks -N, -w)
                nc.vector.scalar_tensor_tensor(
                    out=scr[:], in0=ps[:], scalar=0.0, in1=mask_t[:],
                    op0=Alu.add, op1=Alu.mult,
                    accum_out=Scol[:, b:b + 1])

        emit_load(0)
        emit_load(1)
        emit_w(0)
        emit_act(0)
        for g in range(NG):
            if g + 1 < NG:
                emit_w(g + 1)
            emit_tail(g)
            if g >= 1:
                emit_diag(g - 1)
            if g + 2 < NG:
                emit_load(g + 2)
            if g + 1 < NG:
                emit_act(g + 1)
        emit_diag(NG - 1)

        # Ship per-partition partials; host does the 128-way sum.
        nc.sync.dma_start(out=out_t.ap(), in_=Scol[:])

    nc.compile()
    return nc


def _get_nc():
    if "nc" not in _GLOBAL:
        _GLOBAL["nc"] = _build()
    return _GLOBAL["nc"]


def _mask_np():
    m = np.zeros((P, 2 * P), dtype=np.float32)
    idx = np.arange(P)
    m[idx, idx] = -1.0          # -(F' * N) = F * N
    m[idx, P + idx] = -1.0      # -(F' * w) = F * w
    return m


GAMMA = 2.0
ALPHA = 0.25
SIZE_POWER = 0.5


def kernel(pred: np.ndarray, target: np.ndarray) -> np.ndarray:
    from concourse import bass_utils

    nc = _get_nc()
    pred = np.ascontiguousarray(np.asarray(pred, dtype=np.float32))
    target = np.ascontiguousarray(np.asarray(target, dtype=np.int32))
    mask = _mask_np()

    in_maps = []
    for i in range(N_CORES):
        sl = slice(i * B_PER_CORE, (i + 1) * B_PER_CORE)
        in_maps.append({
            "pred": np.ascontiguousarray(pred[sl, 0]),
            "target": np.ascontiguousarray(target[sl]),
            "mask": mask,
        })

    res = bass_utils.run_bass_kernel_spmd(
        nc, in_maps, core_ids=list(range(N_CORES)),
        trace=bool(_GLOBAL.get("trace", False)),
        **_GLOBAL.get("run_kwargs", {}),
    )
    _GLOBAL["last_results"] = res

    outs = np.stack([r["out"] for r in res.results], axis=0)  # [8, 128, 8]
    S = outs.astype(np.float64).sum(axis=1).reshape(-1)  # per-sample sum(a*s2*L)
    fg = np.count_nonzero(target.reshape(target.shape[0], -1), axis=1)
    fg = fg.astype(np.float64)
    sw = np.where(fg > 0,
                  np.minimum(100.0 / np.power(np.maximum(fg, 1.0), SIZE_POWER), 10.0),
                  1.0)
    per_sample = (S / HW) * sw
    return np.float32(per_sample.mean())
